# revision 5
# baseline (speedup 1.0000x reference)
"""DGMoE Trainium2 kernel: dense-gated MoE (router + 8 dense expert MLPs + gated combine).

Sharding: expert-parallel across 8 NeuronCores. Core c holds expert c's MLP
weights and processes ALL tokens through that expert; every core also computes
the (cheap) router in fp32 for all tokens. Each core's g_e-weighted partial
output (plus a g_e column for the denominator) is ReduceScattered over the
token axis, so core c ends up with token shard c of the summed output, which
it normalizes by (sum_e g_e + eps) locally. Host reassembles the 8 shards.

Expert MLP matmuls run in bf16 (fp32 accumulate); router, softmax, gating,
combine and normalization are fp32.
"""

import numpy as np
import ml_dtypes

B, S, D, E, F = 4, 2048, 1024, 8, 4096
N = B * S                      # 8192 tokens
TT = 128                       # tokens per tile
NT = N // TT                   # 64 tiles
NCH = 4                        # ReduceScatter chunks
TPC = NT // NCH                # 16 tiles per chunk
CH_TOK = N // NCH              # 2048 tokens per chunk
SHARD = CH_TOK // 8            # 256 tokens per rank per chunk
PW = 1032                      # p-buffer row width (1024 outs + g_e col + pad)
LAMBDA_SCALE = 0.5
EPS = 1e-6

_cache = {}


def _build():
    import concourse.bass as bass
    import concourse.mybir as mybir
    import concourse.tile as tile
    from concourse import bacc

    dt = mybir.dt
    nc = bacc.Bacc("TRN2", target_bir_lowering=False, debug=False, num_devices=8)

    xT = nc.dram_tensor("xT", [D, N], dt.float32, kind="ExternalInput")
    plT = nc.dram_tensor("plT", [E, N], dt.float32, kind="ExternalInput")
    WtT = nc.dram_tensor("WtT", [D, E], dt.float32, kind="ExternalInput")
    WgtT = nc.dram_tensor("WgtT", [E, E], dt.float32, kind="ExternalInput")
    thr = nc.dram_tensor("thr", [128, E], dt.float32, kind="ExternalInput")
    esel = nc.dram_tensor("esel", [128, E], dt.float32, kind="ExternalInput")
    W1 = nc.dram_tensor("W1", [D, F], dt.bfloat16, kind="ExternalInput")
    W2 = nc.dram_tensor("W2", [F, D], dt.bfloat16, kind="ExternalInput")
    b1 = nc.dram_tensor("b1", [128, F // 128], dt.float32, kind="ExternalInput")
    b2 = nc.dram_tensor("b2", [1, D], dt.bfloat16, kind="ExternalInput")

    yout = nc.dram_tensor("yout", [N // 8, D], dt.float32, kind="ExternalOutput")
    lgout = nc.dram_tensor("lgout", [N, E], dt.float32, kind="ExternalOutput")

    DC = D // 128   # 8 d-chunks
    FC = F // 128   # 32 f-chunks

    with tile.TileContext(nc) as tc:
        with (
            tc.tile_pool(name="const", bufs=1) as cpool,
            tc.tile_pool(name="work", bufs=2) as wpool,
            tc.tile_pool(name="small", bufs=3) as spool,
            tc.tile_pool(name="psA", bufs=4, space="PSUM") as psA,
            tc.tile_pool(name="psB", bufs=2, space="PSUM") as psB,
            tc.tile_pool(name="psR", bufs=2, space="PSUM") as psR,
            tc.tile_pool(name="dram", bufs=1, space="DRAM") as dpool,
        ):
            # ---- constants / weights resident in SBUF ----
            w1s = cpool.tile([128, DC, F], dt.bfloat16)
            nc.sync.dma_start(w1s[:], W1.ap().rearrange("(c p) f -> p c f", p=128))
            w2s = cpool.tile([128, FC, D], dt.bfloat16)
            nc.sync.dma_start(w2s[:], W2.ap().rearrange("(c p) d -> p c d", p=128))
            wts = cpool.tile([128, DC, E], dt.float32)
            nc.sync.dma_start(wts[:], WtT.ap().rearrange("(c p) e -> p c e", p=128))
            wgts = cpool.tile([E, E], dt.float32)
            nc.sync.dma_start(wgts[:], WgtT.ap())
            thrs = cpool.tile([128, E], dt.float32)
            nc.sync.dma_start(thrs[:], thr.ap())
            esels = cpool.tile([128, E], dt.float32)
            nc.sync.dma_start(esels[:], esel.ap())
            b1s = cpool.tile([128, F // 128], dt.float32)
            nc.sync.dma_start(b1s[:], b1.ap())
            b2s = cpool.tile([1, D], dt.bfloat16)
            nc.sync.dma_start(b2s[:1, :], b2.ap())
            ones = cpool.tile([1, 128], dt.bfloat16)
            nc.vector.memset(ones[:1, :], 1.0)

            # ---- DRAM buffers for the collective ----
            pbufs = []
            rsouts = []
            for k in range(NCH):
                pb = dpool.tile([CH_TOK, PW], dt.float32, name=f"pbuf{k}")
                ro = dpool.tile([SHARD, PW], dt.float32, name=f"rsout{k}")
                pbufs.append(pb)
                rsouts.append(ro)

            xT_r = xT.ap().rearrange("(c p) t -> p c t", p=128)

            for ch in range(NCH):
                for it in range(TPC):
                    tt = ch * TPC + it
                    t0 = tt * TT

                    xf = wpool.tile([128, DC, TT], dt.float32, tag="xf")
                    nc.sync.dma_start(xf[:], xT_r[:, :, t0:t0 + TT])
                    plt = wpool.tile([E, TT], dt.float32, tag="plt")
                    nc.sync.dma_start(plt[:E, :], plT.ap()[:, t0:t0 + TT])
                    xb = wpool.tile([128, DC, TT], dt.bfloat16, tag="xb")
                    nc.vector.tensor_copy(xb[:], xf[:])

                    # ---- router (fp32, exact) ----
                    lps = psR.tile([128, E], dt.float32, tag="lps")
                    for dc in range(DC):
                        nc.tensor.matmul(lps[:], xf[:, dc, :], wts[:, dc, :],
                                         start=(dc == 0), stop=False)
                    nc.tensor.matmul(lps[:], plt[:E, :], wgts[:E, :],
                                     start=False, stop=True)
                    lg = spool.tile([128, E], dt.float32, tag="lg")
                    nc.scalar.copy(lg[:], lps[:])
                    nc.sync.dma_start(lgout.ap()[t0:t0 + TT, :], lg[:])

                    ex = spool.tile([128, E], dt.float32, tag="ex")
                    nc.scalar.activation(ex[:], lps[:],
                                         mybir.ActivationFunctionType.Exp)
                    ssum = spool.tile([128, 1], dt.float32, tag="ssum")
                    nc.vector.reduce_sum(ssum[:], ex[:], mybir.AxisListType.X)
                    rcp = spool.tile([128, 1], dt.float32, tag="rcp")
                    nc.vector.reciprocal(rcp[:], ssum[:])
                    st = spool.tile([128, E], dt.float32, tag="st")
                    nc.vector.tensor_scalar_mul(st[:], ex[:], rcp[:])
                    mask = spool.tile([128, E], dt.float32, tag="mask")
                    nc.vector.tensor_tensor(mask[:], st[:], thrs[:],
                                            op=mybir.AluOpType.is_gt)
                    g = spool.tile([128, E], dt.float32, tag="g")
                    nc.vector.tensor_mul(g[:], st[:], mask[:])
                    gsel = spool.tile([128, E], dt.float32, tag="gsel")
                    nc.vector.tensor_mul(gsel[:], g[:], esels[:])
                    ge = spool.tile([128, 1], dt.float32, tag="ge")
                    nc.vector.reduce_sum(ge[:], gsel[:], mybir.AxisListType.X)

                    # ---- phase 1: hT[f, t] = gelu(W1.T x + b1), bf16 ----
                    ht = wpool.tile([128, FC, TT], dt.bfloat16, tag="ht")
                    for fc in range(FC):
                        hp = psA.tile([128, TT], dt.float32, tag="hp")
                        for dc in range(DC):
                            nc.tensor.matmul(
                                hp[:], w1s[:, dc, fc * 128:(fc + 1) * 128],
                                xb[:, dc, :],
                                start=(dc == 0), stop=(dc == DC - 1))
                        nc.scalar.activation(ht[:, fc, :], hp[:],
                                             mybir.ActivationFunctionType.Gelu,
                                             bias=b1s[:, fc:fc + 1])

                    # ---- phase 2: yp[t, d] = hT.T @ W2 + b2, scaled by g_e ----
                    pout = wpool.tile([128, PW], dt.float32, tag="pout")
                    for dh in range(2):
                        yp = psB.tile([128, 512], dt.float32, tag="yp")
                        for fc in range(FC):
                            nc.tensor.matmul(
                                yp[:], ht[:, fc, :],
                                w2s[:, fc, dh * 512:(dh + 1) * 512],
                                start=(fc == 0), stop=False)
                        nc.tensor.matmul(yp[:], ones[:1, :],
                                         b2s[:1, dh * 512:(dh + 1) * 512],
                                         start=False, stop=True)
                        nc.vector.tensor_scalar_mul(
                            pout[:, dh * 512:(dh + 1) * 512], yp[:], ge[:])
                    nc.vector.tensor_copy(pout[:, 1024:1025], ge[:])
                    nc.vector.memset(pout[:, 1025:PW], 0.0)
                    nc.sync.dma_start(pbufs[ch][it * TT:(it + 1) * TT, :], pout[:])

                # ---- ReduceScatter this chunk over the token axis ----
                nc.gpsimd.collective_compute(
                    "ReduceScatter",
                    mybir.AluOpType.add,
                    replica_groups=[list(range(8))],
                    ins=[pbufs[ch][:].opt()],
                    outs=[rsouts[ch][:].opt()],
                )

                # ---- normalize own shard: y = rs[:, :1024]/(rs[:,1024]+eps) ----
                for s in range(SHARD // 128):
                    r0 = s * 128
                    rin = wpool.tile([128, PW], dt.float32, tag="rin")
                    nc.sync.dma_start(rin[:], rsouts[ch][r0:r0 + 128, :])
                    den = spool.tile([128, 1], dt.float32, tag="den")
                    nc.vector.tensor_scalar_add(den[:], rin[:, 1024:1025], EPS)
                    drc = spool.tile([128, 1], dt.float32, tag="drc")
                    nc.vector.reciprocal(drc[:], den[:])
                    yt = wpool.tile([128, D], dt.float32, tag="yt")
                    nc.vector.tensor_scalar_mul(yt[:], rin[:, 0:1024], drc[:])
                    nc.sync.dma_start(
                        yout.ap()[ch * SHARD + r0:ch * SHARD + r0 + 128, :], yt[:])

    nc.compile()
    return nc


def prepare_in_maps(x, prev_logits, Wt, Wgt, We_logits, W1, b1, W2, b2):
    x2 = np.ascontiguousarray(np.asarray(x, dtype=np.float32).reshape(N, D))
    xT = np.ascontiguousarray(x2.T)
    plT = np.ascontiguousarray(
        np.asarray(prev_logits, dtype=np.float32).reshape(N, E).T)
    WtT = np.ascontiguousarray(np.asarray(Wt, dtype=np.float32).T)
    WgtT = np.ascontiguousarray(np.asarray(Wgt, dtype=np.float32).T)
    thr_v = (LAMBDA_SCALE / (1.0 + np.exp(-np.asarray(We_logits, np.float64)))
             ).astype(np.float32)
    thr_b = np.ascontiguousarray(np.tile(thr_v[None, :], (128, 1)))

    W1 = np.asarray(W1); W2 = np.asarray(W2)
    b1 = np.asarray(b1); b2 = np.asarray(b2)

    in_maps = []
    for c in range(8):
        sel = np.zeros((128, E), np.float32)
        sel[:, c] = 1.0
        in_maps.append({
            "xT": xT,
            "plT": plT,
            "WtT": WtT,
            "WgtT": WgtT,
            "thr": thr_b,
            "esel": sel,
            "W1": np.ascontiguousarray(W1[c].astype(ml_dtypes.bfloat16)),
            "W2": np.ascontiguousarray(W2[c].astype(ml_dtypes.bfloat16)),
            "b1": np.ascontiguousarray(
                b1[c].astype(np.float32).reshape(F // 128, 128).T),
            "b2": np.ascontiguousarray(
                b2[c].astype(ml_dtypes.bfloat16).reshape(1, D)),
        })
    return in_maps


def assemble(results):
    y = np.empty((N, D), np.float32)
    for k in range(NCH):
        for r in range(8):
            y[k * CH_TOK + r * SHARD:k * CH_TOK + (r + 1) * SHARD] = \
                results[r]["yout"][k * SHARD:(k + 1) * SHARD]
    logits = results[0]["lgout"]
    return (y.reshape(B, S, D), logits.reshape(B, S, E))


def kernel(x, prev_logits, Wt, Wgt, We_logits, W1, b1, W2, b2):
    from concourse.bass_utils import run_bass_kernel_spmd

    if "nc" not in _cache:
        _cache["nc"] = _build()
    nc = _cache["nc"]
    in_maps = prepare_in_maps(x, prev_logits, Wt, Wgt, We_logits, W1, b1, W2, b2)
    res = run_bass_kernel_spmd(nc, in_maps, core_ids=list(range(8)))
    _cache["last_results"] = res
    return assemble(res.results)
